# revision 1
# baseline (speedup 1.0000x reference)
"""Trainium2 Bass kernel: 2D valid cross-correlation (4096x4096 image, 15x15 kernel).

Strategy: shard output COLUMNS across 8 NeuronCores (spatial data-parallel,
14-column halo overlap in the input slices; no device-to-device
communication). Each core computes the full 4082 output rows for its 512
output columns. On each core the conv runs on the tensor engine as 15
PSUM-accumulated fp32r matmuls per output tile: for kernel column b, the
stationary operand is the 128x114 banded Toeplitz matrix T_b[r, m] =
w[r - m, b] (contraction over up-to-128 input rows -> up-to-114 output rows)
and the moving operand is the natural row-major X tile offset by b columns in
the free dimension. 36 row-tiles x 15 matmuls of N=511 per core.

fp32r = fp32 rounded to 12 mantissa bits; the PE streams it at 1 column/cycle
(like bf16) while multiplying the rounded values exactly, so the only error
vs the fp32 reference is the input rounding (~1e-4 relative).
"""

import numpy as np

import concourse.bass as bass
import concourse.mybir as mybir
import concourse.tile as tile
from concourse import bacc
from concourse.bass_utils import run_bass_kernel_spmd

H, W = 4096, 4096
KH, KW = 15, 15
OH, OW = H - KH + 1, W - KW + 1  # 4082 x 4082

NCORES = 8
COLS_PER_CORE = 512               # output cols per core (core 7: 498 valid)
IN_COLS = COLS_PER_CORE + KW - 1  # 526 input cols (with halo)

MT = 114                          # output rows per tile (K = MT + 14 = 128)
NT = COLS_PER_CORE                # 512 output cols = one fp32 PSUM bank (2048B)
# NOTE: fp32r matmul requires an EVEN moving free size (ISA s3d3_mm_fp32r)

F32 = mybir.dt.float32
F32R = mybir.dt.float32r

_ROW_TILES = []                   # (row0, M, K)
_r = 0
while _r < OH:
    _m = min(MT, OH - _r)
    _ROW_TILES.append((_r, _m, _m + KH - 1))
    _r += _m
assert _ROW_TILES[-1][0] + _ROW_TILES[-1][2] == H  # 3990 + 106 = 4096


def _build_program():
    nc = bacc.Bacc("TRN2", target_bir_lowering=False, debug=False)
    x = nc.dram_tensor("x", [H, IN_COLS], F32R, kind="ExternalInput").ap()
    wt = nc.dram_tensor("wt", [128, KW * MT], F32R, kind="ExternalInput").ap()
    out = nc.dram_tensor("out", [OH, NT], F32, kind="ExternalOutput").ap()

    with tile.TileContext(nc) as tc:
        with (
            tc.tile_pool(name="wpool", bufs=1) as wpool,
            tc.tile_pool(name="xpool", bufs=12) as xpool,
            tc.tile_pool(name="opool", bufs=3) as opool,
            tc.tile_pool(name="dpool", bufs=1) as dpool,
            tc.tile_pool(name="ppool", bufs=4, space="PSUM") as ppool,
            tc.tile_pool(name="dps", bufs=1, space="PSUM") as dps,
        ):
            # HAM pre-warm: the PE clock-gate only opens to 2.4GHz after
            # ~3.4us of sustained activity. Run dummy matmuls (no input
            # deps) while the first DMAs are in flight so the real matmuls
            # start at full clock.
            dz = dpool.tile([128, 128], F32, tag="dz")
            nc.vector.memset(dz[:], 0)
            dummy = dpool.tile([128, 128], F32R, tag="dummy")
            nc.vector.tensor_copy(dummy[:], dz[:])
            dacc = dps.tile([128, 128], F32)
            for _ in range(40):
                nc.tensor.matmul(dacc[:], dummy[:], dummy[:], start=True, stop=True)

            # Toeplitz pack: b=0 slice as its own tile/DMA so the first real
            # matmul gates on 58KB, not the full 875KB pack.
            wt0 = wpool.tile([128, MT], F32R, tag="wt0")
            nc.scalar.dma_start(wt0[:], wt[:, :MT])
            wtrest = wpool.tile([128, (KW - 1) * MT], F32R, tag="wtr")
            nc.scalar.dma_start(wtrest[:], wt[:, MT:])

            def wslice(b, K, M):
                if b == 0:
                    return wt0[:K, :M]
                return wtrest[:K, (b - 1) * MT : (b - 1) * MT + M]

            out_engines = (nc.gpsimd, nc.scalar)
            for row0, M, K in _ROW_TILES:
                xtile = xpool.tile([128, IN_COLS], F32R, tag="xt")
                nc.sync.dma_start(xtile[:K, :], x[row0 : row0 + K, :])
                acc = ppool.tile([MT, NT], F32)
                for b in range(KW):
                    nc.tensor.matmul(
                        acc[:M, :],
                        wslice(b, K, M),
                        xtile[:K, b : b + NT],
                        start=(b == 0),
                        stop=(b == KW - 1),
                    )
                ot = opool.tile([MT, NT], F32, tag="ot")
                nc.vector.tensor_copy(ot[:M, :], acc[:M, :])
                # Split the store across 2 engine queues (gpsimd+scalar; sync is
                # reserved for input prefetch): a [114, 2048B]
                # tile drains as 2KB-per-partition packets, so one queue
                # adds ~5us of tail latency on the final tile.
                q = (M + 1) // 2
                for e, eng in enumerate(out_engines):
                    p0 = e * q
                    p1 = min(p0 + q, M)
                    if p0 >= p1:
                        break
                    eng.dma_start(
                        out[row0 + p0 : row0 + p1, :], ot[p0:p1, :]
                    )
    nc.finalize()
    return nc


def _round_fp32r(a: np.ndarray) -> np.ndarray:
    """Round fp32 array to fp32r (12 mantissa bits, round-to-nearest-even)."""
    u = np.ascontiguousarray(a, dtype=np.float32).view(np.uint32)
    u = (u + np.uint32(0x7FF) + ((u >> np.uint32(12)) & np.uint32(1))) & np.uint32(
        0xFFFFF000
    )
    return u.view(np.float32)


def _toeplitz_pack(weight: np.ndarray) -> np.ndarray:
    """Pack the 15 banded Toeplitz matrices T_b[r, m] = w[r-m, b] side by side."""
    wt = np.zeros((128, KW * MT), dtype=np.float32)
    r = np.arange(128)[:, None]
    m = np.arange(MT)[None, :]
    a = r - m  # tap index
    valid = (a >= 0) & (a < KH)
    av = np.where(valid, a, 0)
    for b in range(KW):
        wt[:, b * MT : (b + 1) * MT] = np.where(valid, weight[av, b], 0.0)
    return wt


def kernel(X: np.ndarray, weight: np.ndarray, bias: np.ndarray) -> np.ndarray:
    X = np.ascontiguousarray(X, dtype=np.float32)
    weight = np.ascontiguousarray(weight, dtype=np.float32)
    bias = np.asarray(bias, dtype=np.float32)

    Xr = _round_fp32r(X)
    wt = _toeplitz_pack(_round_fp32r(weight))

    in_maps = []
    for c in range(NCORES):
        xs = np.zeros((H, IN_COLS), dtype=np.float32)
        c0 = c * COLS_PER_CORE
        c1 = min(c0 + IN_COLS, W)
        xs[:, : c1 - c0] = Xr[:, c0:c1]
        in_maps.append({"x": xs, "wt": wt})

    nc = _build_program()
    res = run_bass_kernel_spmd(nc, in_maps, core_ids=list(range(NCORES)))
    global _last_results
    _last_results = res

    out = np.empty((OH, OW), dtype=np.float32)
    for c in range(NCORES):
        c0 = c * COLS_PER_CORE
        n = min(COLS_PER_CORE, OW - c0)
        out[:, c0 : c0 + n] = res.results[c]["out"][:, :n]

    b0 = float(bias.reshape(-1)[0]) if bias.size else 0.0
    if b0 != 0.0:
        out += b0
    return out

